# revision 22
# baseline (speedup 1.0000x reference)
"""Trainium2 Bass kernel for fused causal multi-head attention
(qkv projection + causal softmax attention), B=2, T=4096, C=768, nH=12.

Sharding: 8 cores, core c -> batch b=c//4, head group g=c%4 (3 heads each).
Host transposes x[b] to xT [C,T] and column-reorders the weight stack.

Per core (all matmul operands bf16, fp32 PSUM accumulation):
  phase 1: QKV^T projection ([Q^T;K^T] stacked pairwise, V^T)
  phase 1.5: PE-transpose V^T -> V_aug [k,64+1] (ones col = softmax denominator)
  phase 2: attention in S^T orientation, TWO concurrent instruction streams
    on PE row-halves (tile (0,0) vs (64,0)) for 2x matmul throughput:
      S^T[k,q] = K Q^T   (lhsT = K^T 128-col slices, rhs = Q^T 512-col chunk)
      += causal mask (-1e35, DVE)    on diagonal chunks
      P^T = exp(S^T/8)   (ACT, psum[128,<=1536] -> sbuf bf16)
      O^T_aug += V_aug^T P^T  (row 0 = sum of exp = denominator)
    normalize: r = 1/denom row; gpsimd partition_broadcast; DVE multiply.
  Output is O^T [192, T] per core; host transposes into [B,T,C].
"""
import sys
sys.path.insert(0, '/opt/trn_rl_repo')
import numpy as np

import concourse.bass as bass
import concourse.tile as tile
from concourse import bacc, mybir
from concourse import bass_utils

B, T, C, NH = 2, 4096, 768, 12
HD = 64
HPC = 3
NCORES = 8
NQ = T // 512
NKC = T // 128
GRP = 3
NEG = -1.0e35

BF = mybir.dt.bfloat16
CD = BF
F32 = mybir.dt.float32
AF = mybir.ActivationFunctionType
AL = mybir.AluOpType

_CACHE = {}


def _build():
    if 'nc' in _CACHE:
        return _CACHE['nc']
    nc = bacc.Bacc("TRN2", target_bir_lowering=False, debug=False,
                   enable_asserts=True, num_devices=NCORES)
    xT_d = nc.dram_tensor("xT", [C, T], CD, kind="ExternalInput").ap()
    w_d = nc.dram_tensor("w", [C, 576], CD, kind="ExternalInput").ap()
    b_d = nc.dram_tensor("b", [128, 5], F32, kind="ExternalInput").ap()
    out_d = nc.dram_tensor("out", [HPC * HD, T], F32, kind="ExternalOutput").ap()

    # head slot map: (q_tile, q_lo, k_tile, k_lo, v_tile, v_lo)
    # tile0=[Q0;Q1] tile1=[K0;K1] tile2=[Q2;V0] tile3=[K2;V1] tile4=[V2]
    SLOT = [
        (0, 0, 1, 0, 2, 64),
        (0, 64, 1, 64, 3, 64),
        (2, 0, 3, 0, 4, 0),
    ]

    with tile.TileContext(nc) as tc:
        with (
            tc.tile_pool(name="const", bufs=1) as cpool,
            tc.tile_pool(name="persist", bufs=1) as sb,
        ):
            # ---------- constants ----------
            identr_f = cpool.tile([128, 64], F32)
            nc.gpsimd.memset(identr_f[:], 0.0)
            nc.gpsimd.affine_select(out=identr_f[:], in_=identr_f[:],
                                    compare_op=AL.not_equal, fill=1.0, base=0,
                                    channel_multiplier=1, pattern=[[-1, 64]])
            ident_hi_f = cpool.tile([128, 64], F32)
            nc.gpsimd.memset(ident_hi_f[:], 0.0)
            nc.gpsimd.affine_select(out=ident_hi_f[:], in_=ident_hi_f[:],
                                    compare_op=AL.not_equal, fill=1.0, base=-64,
                                    channel_multiplier=1, pattern=[[-1, 64]])
            identr = cpool.tile([128, 64], CD)
            nc.vector.tensor_copy(identr[:], identr_f[:])
            ident_hi = cpool.tile([128, 64], CD)
            nc.vector.tensor_copy(ident_hi[:], ident_hi_f[:])
            ones_f = cpool.tile([128, 1], F32)
            nc.vector.memset(ones_f[:], 1.0)
            masks = cpool.tile([128, 4 * 512], F32)
            nc.gpsimd.memset(masks[:], 0.0)
            for d in range(4):
                nc.gpsimd.affine_select(
                    out=masks[:, d * 512:(d + 1) * 512],
                    in_=masks[:, d * 512:(d + 1) * 512],
                    compare_op=AL.is_ge, fill=NEG,
                    base=-128 * d, channel_multiplier=-1, pattern=[[1, 512]])
            bias_sb = cpool.tile([128, 5], F32)
            nc.sync.dma_start(bias_sb[:], b_d[:])

            qkv_sb = [sb.tile([128, T], CD, name=f"qkv{m}") for m in range(4)]
            qkv_sb.append(sb.tile([64, T], CD, name="qkv4"))
            # duplicates of h2's Q^T/K^T at partitions 64-127 (for the HI stream)
            qdup = sb.tile([128, T], CD, name="qdup")
            kdup = sb.tile([128, T], CD, name="kdup")
            v_aug = [sb.tile([128, NKC * 65], CD, name=f"vaug{h}")
                     for h in range(HPC)]

            # ---------- phase 1: projection ----------
            with (
                tc.tile_pool(name="wsb", bufs=1) as wpool,
                tc.tile_pool(name="xn", bufs=12) as xpool,
                tc.tile_pool(name="pj", bufs=2, space="PSUM") as pjp,
                tc.tile_pool(name="pjsb", bufs=3) as pjp_sb,
            ):
                w_sb = [wpool.tile([128, 576], CD, name=f"w{k}") for k in range(6)]
                for k in range(6):
                    nc.sync.dma_start(w_sb[k][:], w_d[128 * k:128 * (k + 1), :])
                PROJ_PACK = True
                for n in range(NQ):
                    xn = []
                    for k in range(6):
                        t = xpool.tile([128, 512], CD, tag="xn", name=f"xn{n}_{k}")
                        nc.sync.dma_start(
                            t[:], xT_d[128 * k:128 * (k + 1), 512 * n:512 * (n + 1)])
                        xn.append(t)
                    for m in range(5):
                        mw = 128 if m < 4 else 64
                        if PROJ_PACK:
                            pjA = pjp.tile([128, 512], F32, tag="pjA", name=f"pjA{n}_{m}")
                            pjB = pjp.tile([128, 512], F32, tag="pjB", name=f"pjB{n}_{m}")
                            for k in range(6):
                                nc.tensor.matmul(pjA[:mw, :],
                                                 lhsT=w_sb[k][0:64, 128 * m:128 * m + mw],
                                                 rhs=xn[k][0:64, :],
                                                 start=(k == 0), stop=(k == 5))
                                nc.tensor.matmul(pjB[:mw, :],
                                                 lhsT=w_sb[k][64:128, 128 * m:128 * m + mw],
                                                 rhs=xn[k][64:128, :],
                                                 start=(k == 0), stop=(k == 5))
                            dst = qkv_sb[m][:mw, 512 * n:512 * (n + 1)]
                            half = pjp_sb.tile([128, 512], F32, tag="half",
                                               name=f"half{n}_{m}", bufs=3)
                            nc.vector.tensor_scalar(
                                out=half[:mw, :], in0=pjA[:mw, :],
                                scalar1=bias_sb[:mw, m:m + 1],
                                scalar2=None, op0=AL.add)
                            nc.vector.tensor_tensor(
                                out=dst, in0=pjB[:mw, :], in1=half[:mw, :],
                                op=AL.add)
                        else:
                            pj = pjp.tile([128, 512], F32, tag="pj", name=f"pj{n}_{m}")
                            for k in range(6):
                                nc.tensor.matmul(pj[:mw, :],
                                                 lhsT=w_sb[k][:, 128 * m:128 * m + mw],
                                                 rhs=xn[k][:],
                                                 start=(k == 0), stop=(k == 5))
                            nc.vector.tensor_scalar(
                                out=qkv_sb[m][:mw, 512 * n:512 * (n + 1)],
                                in0=pj[:mw, :], scalar1=bias_sb[:mw, m:m + 1],
                                scalar2=None, op0=AL.add)

            # ---------- phase 1.5: V transposes + h2 Q/K duplication ----------
            nc.sync.dma_start(qdup[64:128, :], qkv_sb[2][0:64, :])
            nc.sync.dma_start(kdup[64:128, :], qkv_sb[3][0:64, :])
            with tc.tile_pool(name="vtr", bufs=4, space="PSUM") as vtp:
                for h in range(HPC):
                    vt, vlo = SLOT[h][4], SLOT[h][5]
                    idn = identr if vlo == 0 else ident_hi
                    for i in range(NKC):
                        pt = vtp.tile([128, 64], CD, tag="vt", name=f"vt{h}_{i}")
                        nc.tensor.transpose(
                            pt[:], qkv_sb[vt][vlo:vlo + 64, 128 * i:128 * (i + 1)],
                            idn[vlo:vlo + 64, :])
                        nc.vector.tensor_copy(
                            v_aug[h][:, 65 * i:65 * i + 64], pt[:])
                        nc.vector.tensor_copy(
                            v_aug[h][:, 65 * i + 64:65 * i + 65], ones_f[:])

            # ---------- phase 2: attention, two concurrent streams ----------
            with (
                tc.tile_pool(name="ps_lo", bufs=2, space="PSUM") as pslo,
                tc.tile_pool(name="ps_hi", bufs=1, space="PSUM") as pshi,
                tc.tile_pool(name="pT", bufs=4) as ptp,
                tc.tile_pool(name="nrm", bufs=4) as nrm,
            ):
                def unit(side, h, J, qs_ap, k_tile, k_lo, pspool, potag, prev):
                    """Generator emitting one (head, q-chunk) unit; yields after
                    each PE instruction so two streams can interleave. The final
                    PV flush + normalize is deferred into the NEXT unit (via
                    `prev`) so the PE fills the last exp-wait bubble with the
                    next unit's S^T work."""
                    nK = 4 * (J + 1)
                    po = pspool.tile([65, 512], F32, tag=potag,
                                     name=f"po_{side}_{h}_{J}", bufs=1)
                    pending = None
                    first_group = True
                    for g0 in range(0, nK, GRP):
                        g1 = min(g0 + GRP, nK)
                        wid = 512 * (g1 - g0)
                        ps_s = pspool.tile([128, GRP * 512], F32, tag=f"ps{side}",
                                           name=f"ps_{side}_{h}_{J}_{g0}", bufs=1)
                        for kc in range(g0, g1):
                            sl = slice(512 * (kc - g0), 512 * (kc - g0 + 1))
                            nc.tensor.matmul(
                                ps_s[:, sl],
                                lhsT=k_tile[k_lo:k_lo + 64,
                                            128 * kc:128 * (kc + 1)],
                                rhs=qs_ap, start=True, stop=True)
                            if not GROUP_ILV:
                                yield
                            d = kc - 4 * J
                            if d >= 0:
                                nc.vector.tensor_tensor(
                                    out=ps_s[:, sl], in0=ps_s[:, sl],
                                    in1=masks[:, 512 * d:512 * (d + 1)],
                                    op=AL.add)
                        pT = ptp.tile([128, GRP * 512], CD, tag=f"pT{side}",
                                      name=f"pT_{side}_{h}_{J}_{g0}", bufs=3)
                        nc.scalar.activation(pT[:, :wid], ps_s[:, :wid],
                                             AF.Exp, scale=0.125)
                        if KEEPWARM:
                            nc.tensor.ldweights(identr[0:64, 0:1])
                        if first_group:
                            first_group = False
                            if prev[0] is not None:
                                yield from prev[0]
                                prev[0] = None
                        if pending is not None:
                            pg0, pg1, ppT = pending
                            for kc in range(pg0, pg1):
                                sl = slice(512 * (kc - pg0), 512 * (kc - pg0 + 1))
                                nc.tensor.matmul(
                                    po[:], lhsT=v_aug[h][:, 65 * kc:65 * kc + 65],
                                    rhs=ppT[:, sl],
                                    start=(kc == 0), stop=False)
                                if not GROUP_ILV:
                                    yield
                        pending = (g0, g1, pT)
                        yield

                    def _flush(pg0=pending[0], pg1=pending[1], ppT=pending[2],
                               po=po, h=h, J=J, nK=nK, side=side):
                        for kc in range(pg0, pg1):
                            sl = slice(512 * (kc - pg0), 512 * (kc - pg0 + 1))
                            nc.tensor.matmul(
                                po[:], lhsT=v_aug[h][:, 65 * kc:65 * kc + 65],
                                rhs=ppT[:, sl],
                                start=(kc == 0), stop=(kc == nK - 1))
                            yield
                        # normalize: r = 1/po[64] (denom row); shift to p0;
                        # gpsimd-broadcast; O^T = po[0:64] * r
                        uo = nrm.tile([64, 512], F32, tag="uo",
                                      name=f"uo_{side}_{h}_{J}")
                        nc.vector.tensor_copy(uo[:], po[0:64, :])
                        den = nrm.tile([128, 512], F32, tag="den",
                                       name=f"dn_{side}_{h}_{J}")
                        nc.vector.tensor_copy(den[64:65, :], po[64:65, :])
                        rcph = nrm.tile([128, 512], F32, tag="rcph",
                                        name=f"rh_{side}_{h}_{J}")
                        nc.vector.reciprocal(rcph[64:65, :], den[64:65, :])
                        rcp0 = nrm.tile([1, 512], F32, tag="rcp0",
                                        name=f"rc_{side}_{h}_{J}")
                        nc.sync.dma_start(rcp0[:], rcph[64:65, :])
                        rb = nrm.tile([64, 512], F32, tag="rb",
                                      name=f"rb_{side}_{h}_{J}")
                        nc.gpsimd.partition_broadcast(rb[:], rcp0[:])
                        otn = nrm.tile([64, 512], F32, tag="otn",
                                       name=f"ot_{side}_{h}_{J}")
                        nc.vector.tensor_tensor(out=otn[:], in0=uo[:],
                                                in1=rb[:], op=AL.mult)
                        nc.sync.dma_start(
                            out_d[HD * h:HD * (h + 1), 512 * J:512 * (J + 1)],
                            otn[:])
                    prev[0] = _flush()

                def stream(side, units, pspool, potag):
                    prev = [None]
                    for h, J, qs_ap, k_tile, k_lo in units:
                        yield from unit(side, h, J, qs_ap, k_tile, k_lo,
                                        pspool, potag, prev)
                    if prev[0] is not None:
                        yield from prev[0]

                def qs(tile_idx, lo, J):
                    return qkv_sb[tile_idx][lo:lo + 64, 512 * J:512 * (J + 1)]

                h2_lo = (3, 5, 7) if NQ == 8 else tuple(
                    J for J in range(NQ) if J % 2 == 1)
                h2_hi = tuple(J for J in range(NQ) if J not in h2_lo)
                lo_units = [(0, J, qs(0, 0, J), qkv_sb[1], 0) for J in range(NQ)]
                lo_units += [(2, J, qs(2, 0, J), qkv_sb[3], 0) for J in h2_lo]
                hi_units = [(1, J, qs(0, 64, J), qkv_sb[1], 64) for J in range(NQ)]
                hi_units += [(2, J, qdup[64:128, 512 * J:512 * (J + 1)], kdup, 64)
                             for J in h2_hi]

                GROUP_ILV = False
                KEEPWARM = False
                g_lo = stream("lo", lo_units, pslo, "polo")
                g_hi = stream("hi", hi_units, pshi, "pohi")
                alive_lo = alive_hi = True
                while alive_lo or alive_hi:
                    if alive_lo:
                        try:
                            next(g_lo)
                        except StopIteration:
                            alive_lo = False
                    if alive_hi:
                        try:
                            next(g_hi)
                        except StopIteration:
                            alive_hi = False

    nc.compile()
    _CACHE['nc'] = nc
    return nc


def _prep_inputs(x, w_qkv, b_qkv):
    """Host-side sharding: per-core xT, column-reordered weight stack, bias."""
    import ml_dtypes
    cdt = ml_dtypes.bfloat16
    x = np.asarray(x, dtype=np.float32)
    w_qkv = np.asarray(w_qkv, dtype=np.float32)
    b_qkv = np.asarray(b_qkv, dtype=np.float32)
    xTs = [np.ascontiguousarray(x[b].T).astype(cdt) for b in range(B)]
    in_maps = []
    for c in range(NCORES):
        b_idx, g = c // 4, c % 4
        H = [3 * g, 3 * g + 1, 3 * g + 2]
        q = lambda h: np.arange(64 * h, 64 * (h + 1))
        k = lambda h: np.arange(C + 64 * h, C + 64 * (h + 1))
        v = lambda h: np.arange(2 * C + 64 * h, 2 * C + 64 * (h + 1))
        cols = np.concatenate([
            q(H[0]), q(H[1]),
            k(H[0]), k(H[1]),
            q(H[2]), v(H[0]),
            k(H[2]), v(H[1]),
            v(H[2]),
        ])
        w_stack = np.ascontiguousarray(w_qkv[:, cols]).astype(cdt)
        b_stack = b_qkv[cols]
        bias_pad = np.zeros((128, 5), dtype=np.float32)
        for m in range(4):
            bias_pad[:, m] = b_stack[128 * m:128 * (m + 1)]
        bias_pad[:64, 4] = b_stack[512:576]
        in_maps.append({"xT": xTs[b_idx], "w": w_stack, "b": bias_pad})
    return in_maps


def _run(x, w_qkv, b_qkv, n_head, **run_kwargs):
    assert int(n_head) == NH and x.shape == (B, T, C)
    nc = _build()
    in_maps = _prep_inputs(x, w_qkv, b_qkv)
    res = bass_utils.run_bass_kernel_spmd(
        nc, in_maps, core_ids=list(range(NCORES)), **run_kwargs)
    out = np.empty((B, T, C), dtype=np.float32)
    for c in range(NCORES):
        b_idx, g = c // 4, c % 4
        out[b_idx, :, 192 * g:192 * (g + 1)] = res.results[c]["out"].T
    return out, res


def kernel(x, w_qkv, b_qkv, n_head):
    return _run(x, w_qkv, b_qkv, n_head)[0]


# revision 23
# speedup vs baseline: 1.1087x; 1.1087x over previous
"""Trainium2 Bass kernel for fused causal multi-head attention
(qkv projection + causal softmax attention), B=2, T=4096, C=768, nH=12.

Sharding: 8 cores, core c -> batch b=c//4, head group g=c%4 (3 heads each).
Host transposes x[b] to xT [C,T] and column-reorders the weight stack.

Per core (all matmul operands bf16, fp32 PSUM accumulation):
  phase 1: QKV^T projection ([Q^T;K^T] stacked pairwise, V^T)
  phase 1.5: PE-transpose V^T -> V_aug [k,64+1] (ones col = softmax denominator)
  phase 2: attention in S^T orientation, TWO concurrent instruction streams
    on PE row-halves (tile (0,0) vs (64,0)) for 2x matmul throughput:
      S^T[k,q] = K Q^T   (lhsT = K^T 128-col slices, rhs = Q^T 512-col chunk)
      += causal mask (-1e35, DVE)    on diagonal chunks
      P^T = exp(S^T/8)   (ACT, psum[128,<=1536] -> sbuf bf16)
      O^T_aug += V_aug^T P^T  (row 0 = sum of exp = denominator)
    normalize: r = 1/denom row; gpsimd partition_broadcast; DVE multiply.
  Output is O^T [192, T] per core; host transposes into [B,T,C].
"""
import sys
sys.path.insert(0, '/opt/trn_rl_repo')
import numpy as np

import concourse.bass as bass
import concourse.tile as tile
from concourse import bacc, mybir
from concourse import bass_utils

B, T, C, NH = 2, 4096, 768, 12
HD = 64
HPC = 3
NCORES = 8
NQ = T // 512
NKC = T // 128
GRP = 3
NEG = -1.0e35

BF = mybir.dt.bfloat16
CD = BF
F32 = mybir.dt.float32
AF = mybir.ActivationFunctionType
AL = mybir.AluOpType

_CACHE = {}


def _build():
    if 'nc' in _CACHE:
        return _CACHE['nc']
    nc = bacc.Bacc("TRN2", target_bir_lowering=False, debug=False,
                   enable_asserts=True, num_devices=NCORES)
    xT_d = nc.dram_tensor("xT", [C, T], CD, kind="ExternalInput").ap()
    w_d = nc.dram_tensor("w", [C, 576], CD, kind="ExternalInput").ap()
    b_d = nc.dram_tensor("b", [128, 5], F32, kind="ExternalInput").ap()
    out_d = nc.dram_tensor("out", [HPC * HD, T], F32, kind="ExternalOutput").ap()

    # head slot map: (q_tile, q_lo, k_tile, k_lo, v_tile, v_lo)
    # tile0=[Q0;Q1] tile1=[K0;K1] tile2=[Q2;V0] tile3=[K2;V1] tile4=[V2]
    SLOT = [
        (0, 0, 1, 0, 2, 64),
        (0, 64, 1, 64, 3, 64),
        (2, 0, 3, 0, 4, 0),
    ]

    with tile.TileContext(nc) as tc:
        with (
            tc.tile_pool(name="const", bufs=1) as cpool,
            tc.tile_pool(name="persist", bufs=1) as sb,
        ):
            # ---------- constants ----------
            identr_f = cpool.tile([128, 64], F32)
            nc.gpsimd.memset(identr_f[:], 0.0)
            nc.gpsimd.affine_select(out=identr_f[:], in_=identr_f[:],
                                    compare_op=AL.not_equal, fill=1.0, base=0,
                                    channel_multiplier=1, pattern=[[-1, 64]])
            ident_hi_f = cpool.tile([128, 64], F32)
            nc.gpsimd.memset(ident_hi_f[:], 0.0)
            nc.gpsimd.affine_select(out=ident_hi_f[:], in_=ident_hi_f[:],
                                    compare_op=AL.not_equal, fill=1.0, base=-64,
                                    channel_multiplier=1, pattern=[[-1, 64]])
            identr = cpool.tile([128, 64], CD)
            nc.vector.tensor_copy(identr[:], identr_f[:])
            ident_hi = cpool.tile([128, 64], CD)
            nc.vector.tensor_copy(ident_hi[:], ident_hi_f[:])
            ones_f = cpool.tile([128, 1], F32)
            nc.vector.memset(ones_f[:], 1.0)
            masks = cpool.tile([128, 4 * 512], F32)
            nc.gpsimd.memset(masks[:], 0.0)
            for d in range(4):
                nc.gpsimd.affine_select(
                    out=masks[:, d * 512:(d + 1) * 512],
                    in_=masks[:, d * 512:(d + 1) * 512],
                    compare_op=AL.is_ge, fill=NEG,
                    base=-128 * d, channel_multiplier=-1, pattern=[[1, 512]])
            bias_sb = cpool.tile([128, 5], F32)
            nc.sync.dma_start(bias_sb[:], b_d[:])

            qkv_sb = [sb.tile([128, T], CD, name=f"qkv{m}") for m in range(4)]
            qkv_sb.append(sb.tile([64, T], CD, name="qkv4"))
            # duplicates of h2's Q^T/K^T at partitions 64-127 (for the HI stream)
            qdup = sb.tile([128, T], CD, name="qdup")
            kdup = sb.tile([128, T], CD, name="kdup")
            v_aug = [sb.tile([128, NKC * 65], CD, name=f"vaug{h}")
                     for h in range(HPC)]

            # ---------- phase 1: projection ----------
            with (
                tc.tile_pool(name="wsb", bufs=1) as wpool,
                tc.tile_pool(name="xn", bufs=12) as xpool,
                tc.tile_pool(name="pj", bufs=2, space="PSUM") as pjp,
                tc.tile_pool(name="pjsb", bufs=3) as pjp_sb,
            ):
                w_sb = [wpool.tile([128, 576], CD, name=f"w{k}") for k in range(6)]
                for k in range(6):
                    nc.sync.dma_start(w_sb[k][:], w_d[128 * k:128 * (k + 1), :])
                PROJ_PACK = True
                for n in range(NQ):
                    xn = []
                    for k in range(6):
                        t = xpool.tile([128, 512], CD, tag="xn", name=f"xn{n}_{k}")
                        nc.sync.dma_start(
                            t[:], xT_d[128 * k:128 * (k + 1), 512 * n:512 * (n + 1)])
                        xn.append(t)
                    for m in range(5):
                        mw = 128 if m < 4 else 64
                        if PROJ_PACK:
                            pjA = pjp.tile([128, 512], F32, tag="pjA", name=f"pjA{n}_{m}")
                            pjB = pjp.tile([128, 512], F32, tag="pjB", name=f"pjB{n}_{m}")
                            for k in range(6):
                                nc.tensor.matmul(pjA[:mw, :],
                                                 lhsT=w_sb[k][0:64, 128 * m:128 * m + mw],
                                                 rhs=xn[k][0:64, :],
                                                 start=(k == 0), stop=(k == 5))
                                nc.tensor.matmul(pjB[:mw, :],
                                                 lhsT=w_sb[k][64:128, 128 * m:128 * m + mw],
                                                 rhs=xn[k][64:128, :],
                                                 start=(k == 0), stop=(k == 5))
                            dst = qkv_sb[m][:mw, 512 * n:512 * (n + 1)]
                            half = pjp_sb.tile([128, 512], F32, tag="half",
                                               name=f"half{n}_{m}", bufs=3)
                            nc.vector.tensor_scalar(
                                out=half[:mw, :], in0=pjA[:mw, :],
                                scalar1=bias_sb[:mw, m:m + 1],
                                scalar2=None, op0=AL.add)
                            nc.vector.tensor_tensor(
                                out=dst, in0=pjB[:mw, :], in1=half[:mw, :],
                                op=AL.add)
                        else:
                            pj = pjp.tile([128, 512], F32, tag="pj", name=f"pj{n}_{m}")
                            for k in range(6):
                                nc.tensor.matmul(pj[:mw, :],
                                                 lhsT=w_sb[k][:, 128 * m:128 * m + mw],
                                                 rhs=xn[k][:],
                                                 start=(k == 0), stop=(k == 5))
                            nc.vector.tensor_scalar(
                                out=qkv_sb[m][:mw, 512 * n:512 * (n + 1)],
                                in0=pj[:mw, :], scalar1=bias_sb[:mw, m:m + 1],
                                scalar2=None, op0=AL.add)

            # ---------- phase 1.5: V transposes + h2 Q/K duplication ----------
            nc.sync.dma_start(qdup[64:128, :], qkv_sb[2][0:64, :])
            nc.sync.dma_start(kdup[64:128, :], qkv_sb[3][0:64, :])
            with tc.tile_pool(name="vtr", bufs=4, space="PSUM") as vtp:
                for h in range(HPC):
                    vt, vlo = SLOT[h][4], SLOT[h][5]
                    idn = identr if vlo == 0 else ident_hi
                    for i in range(NKC):
                        pt = vtp.tile([128, 64], CD, tag="vt", name=f"vt{h}_{i}")
                        nc.tensor.transpose(
                            pt[:], qkv_sb[vt][vlo:vlo + 64, 128 * i:128 * (i + 1)],
                            idn[vlo:vlo + 64, :])
                        nc.vector.tensor_copy(
                            v_aug[h][:, 65 * i:65 * i + 64], pt[:])
                        nc.vector.tensor_copy(
                            v_aug[h][:, 65 * i + 64:65 * i + 65], ones_f[:])

            # ---------- phase 2: attention, two concurrent streams ----------
            with (
                tc.tile_pool(name="ps_lo", bufs=2, space="PSUM") as pslo,
                tc.tile_pool(name="ps_hi", bufs=1, space="PSUM") as pshi,
                tc.tile_pool(name="pT", bufs=4) as ptp,
                tc.tile_pool(name="nrm", bufs=4) as nrm,
            ):
                def unit(side, h, J, qs_ap, k_tile, k_lo, pspool, potag):
                    """Generator emitting one (head, q-chunk) unit; yields after
                    each PE instruction so two streams can interleave."""
                    nK = 4 * (J + 1)
                    po = pspool.tile([65, 512], F32, tag=potag,
                                     name=f"po_{side}_{h}_{J}", bufs=1)
                    pending = None
                    for g0 in range(0, nK, GRP):
                        g1 = min(g0 + GRP, nK)
                        wid = 512 * (g1 - g0)
                        ps_s = pspool.tile([128, GRP * 512], F32, tag=f"ps{side}",
                                           name=f"ps_{side}_{h}_{J}_{g0}", bufs=1)
                        for kc in range(g0, g1):
                            sl = slice(512 * (kc - g0), 512 * (kc - g0 + 1))
                            nc.tensor.matmul(
                                ps_s[:, sl],
                                lhsT=k_tile[k_lo:k_lo + 64,
                                            128 * kc:128 * (kc + 1)],
                                rhs=qs_ap, start=True, stop=True)
                            if not GROUP_ILV:
                                yield
                            d = kc - 4 * J
                            if d >= 0:
                                nc.vector.tensor_tensor(
                                    out=ps_s[:, sl], in0=ps_s[:, sl],
                                    in1=masks[:, 512 * d:512 * (d + 1)],
                                    op=AL.add)
                        pT = ptp.tile([128, GRP * 512], CD, tag=f"pT{side}",
                                      name=f"pT_{side}_{h}_{J}_{g0}", bufs=3)
                        nc.scalar.activation(pT[:, :wid], ps_s[:, :wid],
                                             AF.Exp, scale=0.125)
                        if KEEPWARM:
                            nc.tensor.ldweights(identr[0:64, 0:1])
                        if pending is not None:
                            pg0, pg1, ppT = pending
                            for kc in range(pg0, pg1):
                                sl = slice(512 * (kc - pg0), 512 * (kc - pg0 + 1))
                                nc.tensor.matmul(
                                    po[:], lhsT=v_aug[h][:, 65 * kc:65 * kc + 65],
                                    rhs=ppT[:, sl],
                                    start=(kc == 0), stop=False)
                                if not GROUP_ILV:
                                    yield
                        pending = (g0, g1, pT)
                        yield
                    pg0, pg1, ppT = pending
                    for kc in range(pg0, pg1):
                        sl = slice(512 * (kc - pg0), 512 * (kc - pg0 + 1))
                        nc.tensor.matmul(
                            po[:], lhsT=v_aug[h][:, 65 * kc:65 * kc + 65],
                            rhs=ppT[:, sl],
                            start=(kc == 0), stop=(kc == nK - 1))
                        if not GROUP_ILV:
                            yield
                    # normalize: r = 1/po[64] (denom row); shift to p0;
                    # gpsimd-broadcast; O^T = po[0:64] * r
                    uo = nrm.tile([64, 512], F32, tag="uo", name=f"uo_{side}_{h}_{J}")
                    nc.vector.tensor_copy(uo[:], po[0:64, :])
                    den = nrm.tile([128, 512], F32, tag="den", name=f"dn_{side}_{h}_{J}")
                    nc.vector.tensor_copy(den[64:65, :], po[64:65, :])
                    rcph = nrm.tile([128, 512], F32, tag="rcph", name=f"rh_{side}_{h}_{J}")
                    nc.vector.reciprocal(rcph[64:65, :], den[64:65, :])
                    rcp0 = nrm.tile([1, 512], F32, tag="rcp0", name=f"rc_{side}_{h}_{J}")
                    nc.sync.dma_start(rcp0[:], rcph[64:65, :])
                    rb = nrm.tile([64, 512], F32, tag="rb", name=f"rb_{side}_{h}_{J}")
                    nc.gpsimd.partition_broadcast(rb[:], rcp0[:])
                    otn = nrm.tile([64, 512], F32, tag="otn", name=f"ot_{side}_{h}_{J}")
                    nc.vector.tensor_tensor(out=otn[:], in0=uo[:],
                                            in1=rb[:], op=AL.mult)
                    nc.sync.dma_start(
                        out_d[HD * h:HD * (h + 1), 512 * J:512 * (J + 1)],
                        otn[:])

                def stream(side, units, pspool, potag):
                    for h, J, qs_ap, k_tile, k_lo in units:
                        yield from unit(side, h, J, qs_ap, k_tile, k_lo,
                                        pspool, potag)

                def qs(tile_idx, lo, J):
                    return qkv_sb[tile_idx][lo:lo + 64, 512 * J:512 * (J + 1)]

                h2_lo = (3, 5, 7) if NQ == 8 else tuple(
                    J for J in range(NQ) if J % 2 == 1)
                h2_hi = tuple(J for J in range(NQ) if J not in h2_lo)
                lo_units = [(0, J, qs(0, 0, J), qkv_sb[1], 0) for J in range(NQ)]
                lo_units += [(2, J, qs(2, 0, J), qkv_sb[3], 0) for J in h2_lo]
                hi_units = [(1, J, qs(0, 64, J), qkv_sb[1], 64) for J in range(NQ)]
                hi_units += [(2, J, qdup[64:128, 512 * J:512 * (J + 1)], kdup, 64)
                             for J in h2_hi]

                GROUP_ILV = False
                KEEPWARM = False
                g_lo = stream("lo", lo_units, pslo, "polo")
                g_hi = stream("hi", hi_units, pshi, "pohi")
                alive_lo = alive_hi = True
                while alive_lo or alive_hi:
                    if alive_lo:
                        try:
                            next(g_lo)
                        except StopIteration:
                            alive_lo = False
                    if alive_hi:
                        try:
                            next(g_hi)
                        except StopIteration:
                            alive_hi = False

    nc.compile()
    _CACHE['nc'] = nc
    return nc


def _prep_inputs(x, w_qkv, b_qkv):
    """Host-side sharding: per-core xT, column-reordered weight stack, bias."""
    import ml_dtypes
    cdt = ml_dtypes.bfloat16
    x = np.asarray(x, dtype=np.float32)
    w_qkv = np.asarray(w_qkv, dtype=np.float32)
    b_qkv = np.asarray(b_qkv, dtype=np.float32)
    xTs = [np.ascontiguousarray(x[b].T).astype(cdt) for b in range(B)]
    in_maps = []
    for c in range(NCORES):
        b_idx, g = c // 4, c % 4
        H = [3 * g, 3 * g + 1, 3 * g + 2]
        q = lambda h: np.arange(64 * h, 64 * (h + 1))
        k = lambda h: np.arange(C + 64 * h, C + 64 * (h + 1))
        v = lambda h: np.arange(2 * C + 64 * h, 2 * C + 64 * (h + 1))
        cols = np.concatenate([
            q(H[0]), q(H[1]),
            k(H[0]), k(H[1]),
            q(H[2]), v(H[0]),
            k(H[2]), v(H[1]),
            v(H[2]),
        ])
        w_stack = np.ascontiguousarray(w_qkv[:, cols]).astype(cdt)
        b_stack = b_qkv[cols]
        bias_pad = np.zeros((128, 5), dtype=np.float32)
        for m in range(4):
            bias_pad[:, m] = b_stack[128 * m:128 * (m + 1)]
        bias_pad[:64, 4] = b_stack[512:576]
        in_maps.append({"xT": xTs[b_idx], "w": w_stack, "b": bias_pad})
    return in_maps


def _run(x, w_qkv, b_qkv, n_head, **run_kwargs):
    assert int(n_head) == NH and x.shape == (B, T, C)
    nc = _build()
    in_maps = _prep_inputs(x, w_qkv, b_qkv)
    res = bass_utils.run_bass_kernel_spmd(
        nc, in_maps, core_ids=list(range(NCORES)), **run_kwargs)
    out = np.empty((B, T, C), dtype=np.float32)
    for c in range(NCORES):
        b_idx, g = c // 4, c % 4
        out[b_idx, :, 192 * g:192 * (g + 1)] = res.results[c]["out"].T
    return out, res


def kernel(x, w_qkv, b_qkv, n_head):
    return _run(x, w_qkv, b_qkv, n_head)[0]
